# revision 15
# baseline (speedup 1.0000x reference)
"""Bidirectional Mamba block on 8 Trainium2 NeuronCores.

Sharding: tensor-parallel over d_inner (2048 -> 256 per core). Each core:
  - computes its x/z in_proj slice from the full hidden (replicated, bf16)
  - depthwise causal conv + silu for both directions (reverse direction is
    computed in flipped coordinates "s-space" so no data flips are needed;
    the reverse scan runs via negative-stride access patterns)
  - x-projection partials -> AllReduce -> full (dt, B, C) per core
  - selective scan per (direction, batch, d-tile, state-index) with the
    hardware tensor_tensor_scan instruction
  - gating + out_proj partial -> ReduceScatter over L-stripes
Host gathers the 8 L-stripes and transposes to (B, L, D_MODEL).

softplus is unavailable in the ACT tables; we use the exact identities
  G = sigmoid(-(x+b)),  delta = softplus(x+b) = -ln G,
  dA_n = exp(A_n * delta) = exp(-A_n * ln G)
so only Sigmoid/Ln/Exp/Silu/Copy LUTs are needed.
"""

from contextlib import ExitStack

import numpy as np
import ml_dtypes

B, L, DM = 2, 2048, 1024
DI, N, DCONV, R = 2048, 16, 4, 64
NCORES = 8
DC = DI // NCORES          # 256 d_inner channels per core
TD = DC // 128             # 2 d-tiles of 128
TC = 512                   # t-chunk for matmuls
NTC = L // TC
BF = ml_dtypes.bfloat16

_CACHE = {}


def _rev(ap):
    """Reverse the (single) free dim of a 2D [partition, free] AP."""
    import concourse.bass as bass
    n = ap.ap[-1][1]
    assert ap.ap[-1][0] == 1
    return bass.AP(ap.tensor, ap.offset + (n - 1),
                   [list(d) for d in ap.ap[:-1]] + [[-1, n]])


def _bcast(ap, parts=128):
    """Partition-broadcast a DRAM row (1D AP of length F) to (parts, F)."""
    import concourse.bass as bass
    assert ap.ap[-1][0] == 1
    return bass.AP(ap.tensor, ap.offset, [[0, parts], [1, ap.ap[-1][1]]])


def _build():
    import concourse.tile as tile
    from concourse import bacc, mybir
    from concourse.masks import make_identity

    f32, bf16 = mybir.dt.float32, mybir.dt.bfloat16
    AF = mybir.ActivationFunctionType
    OP = mybir.AluOpType

    nc = bacc.Bacc("TRN2", target_bir_lowering=False, debug=False,
                   num_devices=NCORES)

    # ---------------- DRAM parameters (per-core shards, host-prepped)
    hT = nc.declare_dram_parameter("hT", [DM, B * L], bf16, isOutput=False)
    win = nc.declare_dram_parameter("win", [DM, 4 * 128], bf16, isOutput=False)
    wx = nc.declare_dram_parameter("wx", [2, DC, 96], bf16, isOutput=False)
    wdt = nc.declare_dram_parameter("wdt", [2, R, DC], f32, isOutput=False)
    wout = nc.declare_dram_parameter("wout", [DC, DM], bf16, isOutput=False)
    wconv = nc.declare_dram_parameter("wconv", [2, DCONV, DC], f32, isOutput=False)
    cbias = nc.declare_dram_parameter("cbias", [2, DC], f32, isOutput=False)
    bdt = nc.declare_dram_parameter("bdt", [2, DC], f32, isOutput=False)  # = -b_dt
    Dp = nc.declare_dram_parameter("Dp", [2, DC], f32, isOutput=False)
    Amat = nc.declare_dram_parameter("Amat", [2, DC, N], f32, isOutput=False)  # = -A
    out_p = nc.declare_dram_parameter("out", [B, DM, L // NCORES], f32,
                                      isOutput=True)

    # ---------------- internal DRAM
    xdp = nc.dram_tensor("xdp", [2, B, 96, L], f32)
    xdr = nc.dram_tensor("xdr", [2, B, 96, L], f32, addr_space="Shared")
    zdram = nc.dram_tensor("zdram", [B * TD, 128, L], bf16)
    udram = nc.dram_tensor("udram", [2, B * TD, 128, L], bf16)
    bcbf = nc.dram_tensor("bcbf", [2, B, 32, L], bf16)
    po = nc.dram_tensor("po", [NCORES, B, DM, L // NCORES], f32)
    rso = nc.dram_tensor("rso", [B, DM, L // NCORES], f32)

    with tile.TileContext(nc) as tc, ExitStack() as es:
        ws = es.enter_context(tc.tile_pool(name="weights", bufs=1))
        wx_sb = ws.tile([128, 2 * TD, 96], bf16)
        nc.sync.dma_start(out=wx_sb[:], in_=wx[:].rearrange("d (t p) m -> p (d t) m", p=128))
        wdt_sb = ws.tile([R, 2, DC], f32)
        nc.sync.dma_start(out=wdt_sb[:], in_=wdt[:].rearrange("d k m -> k d m"))
        wout_sb = ws.tile([128, TD, DM], bf16)
        nc.sync.dma_start(out=wout_sb[:], in_=wout[:].rearrange("(t p) m -> p t m", p=128))
        wconv_sb = ws.tile([128, 2, DCONV, TD], f32)
        nc.sync.dma_start(out=wconv_sb[:], in_=wconv[:].rearrange("d k (t p) -> p d k t", p=128))
        cbias_sb = ws.tile([128, 2, TD], f32)
        nc.sync.dma_start(out=cbias_sb[:], in_=cbias[:].rearrange("d (t p) -> p d t", p=128))
        bdt_sb = ws.tile([128, 2, TD], f32)
        nc.sync.dma_start(out=bdt_sb[:], in_=bdt[:].rearrange("d (t p) -> p d t", p=128))
        Dp_sb = ws.tile([128, 2, TD], f32)
        nc.sync.dma_start(out=Dp_sb[:], in_=Dp[:].rearrange("d (t p) -> p d t", p=128))
        A_sb = ws.tile([128, 2, TD, N], f32)
        nc.sync.dma_start(out=A_sb[:], in_=Amat[:].rearrange("d (t p) n -> p d t n", p=128))

        # persistent SBUF: only comb (out_proj input)
        act = es.enter_context(tc.tile_pool(name="acts", bufs=1))
        comb_sb = [act.tile([128, L], bf16, name=f"comb{g}") for g in range(B * TD)]

        # ---------------- Phase 0/1: in_proj + conv + silu -> udram/zdram
        with tc.tile_pool(name="p01", bufs=2) as p01, \
             tc.tile_pool(name="x01", bufs=1) as x01, \
             tc.tile_pool(name="ps01", bufs=4, space="PSUM") as ps01:
            win_sb = x01.tile([128, 8, 512], bf16)
            nc.sync.dma_start(out=win_sb[:], in_=win[:].rearrange("(k p) m -> p k m", p=128))
            x_sb = [x01.tile([128, L], bf16, name=f"x{g}") for g in range(B * TD)]
            for b in range(B):
                for tcn in range(NTC):
                    hTc = p01.tile([128, 8, TC], bf16, name="hTc")
                    nc.sync.dma_start(
                        out=hTc[:],
                        in_=hT[:, b * L + tcn * TC: b * L + (tcn + 1) * TC]
                        .rearrange("(k p) c -> p k c", p=128))
                    for part in range(4):   # x-dt0, x-dt1, z-dt0, z-dt1
                        pst = ps01.tile([128, TC], f32, name="pst")
                        for k in range(8):
                            nc.tensor.matmul(pst[:], win_sb[:, k, part * 128:(part + 1) * 128],
                                             hTc[:, k, :], start=(k == 0), stop=(k == 7))
                        if part < TD:
                            nc.scalar.copy(
                                out=x_sb[b * TD + part][:, tcn * TC:(tcn + 1) * TC],
                                in_=pst[:])
                        else:
                            zt = p01.tile([128, TC], bf16, name="zt")
                            nc.scalar.activation(out=zt[:], in_=pst[:], func=AF.Silu)
                            nc.sync.dma_start(
                                out=zdram[b * TD + part - TD, :,
                                          tcn * TC:(tcn + 1) * TC],
                                in_=zt[:])
            # depthwise causal conv + silu, both directions
            for d in range(2):
                for g in range(B * TD):
                    ti = g % TD
                    xs = x_sb[g]
                    acc = p01.tile([128, L], bf16, name="acc")
                    if d == 0:  # causal: acc[t] = sum_k w[k]*x[t-3+k] + cb
                        nc.vector.tensor_scalar(
                            out=acc[:], in0=xs[:],
                            scalar1=wconv_sb[:, 0, 3, ti:ti + 1],
                            scalar2=cbias_sb[:, 0, ti:ti + 1],
                            op0=OP.mult, op1=OP.add)
                        for k in (2, 1, 0):
                            s = 3 - k
                            nc.vector.scalar_tensor_tensor(
                                out=acc[:, s:], in0=xs[:, :L - s],
                                scalar=wconv_sb[:, 0, k, ti:ti + 1], in1=acc[:, s:],
                                op0=OP.mult, op1=OP.add)
                    else:  # reverse dir in s-space: acc[s] = sum_m wr[3-m]*x[s+m] + cbr
                        nc.vector.tensor_scalar(
                            out=acc[:], in0=xs[:],
                            scalar1=wconv_sb[:, 1, 3, ti:ti + 1],
                            scalar2=cbias_sb[:, 1, ti:ti + 1],
                            op0=OP.mult, op1=OP.add)
                        for m in (1, 2, 3):
                            nc.vector.scalar_tensor_tensor(
                                out=acc[:, :L - m], in0=xs[:, m:],
                                scalar=wconv_sb[:, 1, 3 - m, ti:ti + 1],
                                in1=acc[:, :L - m],
                                op0=OP.mult, op1=OP.add)
                    ut = p01.tile([128, L], bf16, name="ut")
                    nc.scalar.activation(out=ut[:], in_=acc[:], func=AF.Silu)
                    nc.sync.dma_start(out=udram[d, g], in_=ut[:])

        # ---------------- Phase 2: x-proj partials + AllReduce
        with tc.tile_pool(name="p2", bufs=2) as p2, \
             tc.tile_pool(name="ps2", bufs=4, space="PSUM") as ps2:
            for d in range(2):
                for b in range(B):
                    uts = []
                    for kt in range(TD):
                        ut2 = p2.tile([128, L], bf16, name=f"ut2_{kt}", tag=f"ut2_{kt}")
                        nc.sync.dma_start(out=ut2[:], in_=udram[d, b * TD + kt])
                        uts.append(ut2)
                    for tcn in range(NTC):
                        ps96 = ps2.tile([96, TC], f32, name="ps96")
                        for kt in range(TD):
                            nc.tensor.matmul(ps96[:], wx_sb[:, d * TD + kt, :],
                                             uts[kt][:, tcn * TC:(tcn + 1) * TC],
                                             start=(kt == 0), stop=(kt == TD - 1))
                        sb96 = p2.tile([96, TC], f32, name="sb96")
                        nc.scalar.copy(out=sb96[:], in_=ps96[:])
                        nc.sync.dma_start(
                            out=xdp[d, b, :, tcn * TC:(tcn + 1) * TC], in_=sb96[:])
        # split AllReduce: (b0,d0) first so phase 3 can start early
        for b in range(B):
            for d in range(2):
                nc.gpsimd.collective_compute(
                    "AllReduce", OP.add, replica_groups=[list(range(NCORES))],
                    ins=[xdp[d, b].opt()], outs=[xdr[d, b].opt()])

        # ---------------- Phase 3: delta, scan, gating
        idn = ws.tile([128, 128], bf16, name="idn")
        make_identity(nc, idn[:])
        with tc.tile_pool(name="p3", bufs=1) as p3, \
             tc.tile_pool(name="pbc", bufs=4) as pbc, \
             tc.tile_pool(name="pda", bufs=3) as pda, \
             tc.tile_pool(name="ph", bufs=3) as ph, \
             tc.tile_pool(name="phc", bufs=3) as phc, \
             tc.tile_pool(name="ps3", bufs=2, space="PSUM") as ps3:
            for b in range(B):
                for d in range(2):
                    dtT = p3.tile([R, L], f32, name="dtT", bufs=1)
                    nc.sync.dma_start(out=dtT[:], in_=xdr[d, b, 0:R, :])
                    # stage B/C rows to bf16 (cheap bcast sources)
                    bcst = p3.tile([32, L], f32, name="bcst", bufs=1)
                    nc.sync.dma_start(out=bcst[:], in_=xdr[d, b, 64:96, :])
                    bcsb = p3.tile([32, L], bf16, name="bcsb", bufs=1)
                    nc.scalar.copy(out=bcsb[:], in_=bcst[:])
                    nc.sync.dma_start(out=bcbf[d, b], in_=bcsb[:])
                    lgs, dus, u3s, yaccs = [], [], [], []
                    for ti in range(TD):
                        u3 = p3.tile([128, L], bf16, name=f"u3_{ti}", bufs=1)
                        nc.sync.dma_start(out=u3[:], in_=udram[d, b * TD + ti])
                        u3s.append(u3)
                        # G = sigmoid(-(dtproj + b_dt)); delta = -ln G
                        lg = p3.tile([128, L], f32, name=f"lg{ti}", bufs=1)
                        for tcn in range(NTC):
                            psd = ps3.tile([128, TC], f32, name="psd", tag="ps",
                                           padded_shape=[128, L])
                            nc.tensor.matmul(psd[:], wdt_sb[:, d, ti * 128:(ti + 1) * 128],
                                             dtT[:, tcn * TC:(tcn + 1) * TC],
                                             start=True, stop=True)
                            nc.scalar.copy(out=lg[:, tcn * TC:(tcn + 1) * TC], in_=psd[:])
                        nc.scalar.activation(out=lg[:], in_=lg[:], func=AF.Sigmoid,
                                             scale=-1.0, bias=bdt_sb[:, d, ti:ti + 1])
                        nc.scalar.activation(out=lg[:], in_=lg[:], func=AF.Ln)
                        lgs.append(lg)
                        du = p3.tile([128, L], bf16, name=f"du{ti}", bufs=1)
                        nc.vector.scalar_tensor_tensor(
                            out=du[:], in0=lg[:], scalar=-1.0, in1=u3[:],
                            op0=OP.mult, op1=OP.mult)
                        dus.append(du)
                        yacc = ps3.tile([128, L], f32, name=f"yacc{ti}", tag="ps")
                        yaccs.append(yacc)
                    # scan over the 16 states; PE-accumulated reduction over n
                    for n in range(N):
                        Bbc = pbc.tile([128, L], bf16, name="Bbc")
                        nc.sync.dma_start(out=Bbc[:], in_=_bcast(bcbf[d, b, n, :]))
                        Cbc = pbc.tile([128, L], bf16, name="Cbc")
                        nc.sync.dma_start(out=Cbc[:], in_=_bcast(bcbf[d, b, 16 + n, :]))
                        for ti in range(TD):
                            dA = pda.tile([128, L], f32, name="dA", bufs=4)
                            nc.scalar.activation(out=dA[:], in_=lgs[ti][:], func=AF.Exp,
                                                 scale=A_sb[:, d, ti, n:n + 1])
                            dBu = pda.tile([128, L], bf16, name="dBu", bufs=3)
                            nc.vector.tensor_mul(out=dBu[:], in0=dus[ti][:], in1=Bbc[:])
                            Ht = ph.tile([128, L], bf16, name="Ht")
                            if d == 0:
                                nc.vector.tensor_tensor_scan(
                                    out=Ht[:], data0=dA[:], data1=dBu[:], initial=0.0,
                                    op0=OP.mult, op1=OP.add)
                            else:
                                nc.vector.tensor_tensor_scan(
                                    out=_rev(Ht[:]), data0=_rev(dA[:]), data1=_rev(dBu[:]),
                                    initial=0.0, op0=OP.mult, op1=OP.add)
                            Hc = phc.tile([128, L], bf16, name="Hc")
                            eng_hc = nc.gpsimd
                            eng_hc.tensor_mul(out=Hc[:], in0=Ht[:], in1=Cbc[:])
                            for ch in range(NTC):
                                nc.tensor.matmul(
                                    yaccs[ti][:, ch * TC:(ch + 1) * TC], idn[:],
                                    Hc[:, ch * TC:(ch + 1) * TC],
                                    start=(n == 0), stop=(n == N - 1))
                    for ti in range(TD):
                        g = b * TD + ti
                        # gating: comb = (u*D + y) * silu(z)   (zdram holds silu(z))
                        y32 = p3.tile([128, L], f32, name="y32", bufs=1)
                        nc.scalar.copy(out=y32[:], in_=yaccs[ti][:])
                        zt3 = p3.tile([128, L], bf16, name="zt3", bufs=1)
                        nc.sync.dma_start(out=zt3[:], in_=zdram[g])
                        t1 = p3.tile([128, L], bf16, name="t1", bufs=1)
                        nc.vector.scalar_tensor_tensor(
                            out=t1[:], in0=u3s[ti][:], scalar=Dp_sb[:, d, ti:ti + 1],
                            in1=y32[:], op0=OP.mult, op1=OP.add)
                        if d == 0:
                            nc.vector.tensor_mul(out=comb_sb[g][:], in0=t1[:], in1=zt3[:])
                        else:
                            yg = p3.tile([128, L], bf16, name="yg", bufs=1)
                            nc.vector.tensor_mul(out=yg[:], in0=t1[:], in1=zt3[:])
                            nc.vector.tensor_add(out=comb_sb[g][:], in0=comb_sb[g][:],
                                                 in1=yg[:])

        # ---------------- Phase 4: out_proj + ReduceScatter
        LS = L // NCORES  # 256
        with tc.tile_pool(name="ps4", bufs=4, space="PSUM") as ps4, \
             tc.tile_pool(name="p4s", bufs=3) as p4s:
            for b in range(B):
                for mt in range(DM // 128):
                    for tcn in range(NTC):
                        pso = ps4.tile([128, TC], f32, name="pso")
                        for kt in range(TD):
                            nc.tensor.matmul(
                                pso[:], wout_sb[:, kt, mt * 128:(mt + 1) * 128],
                                comb_sb[b * TD + kt][:, tcn * TC:(tcn + 1) * TC],
                                start=(kt == 0), stop=(kt == TD - 1))
                        sbo = p4s.tile([128, TC], f32, name="sbo")
                        nc.scalar.copy(out=sbo[:], in_=pso[:])
                        for half in range(TC // LS):
                            r = tcn * (TC // LS) + half
                            nc.sync.dma_start(
                                out=po[r, b, mt * 128:(mt + 1) * 128, :],
                                in_=sbo[:, half * LS:(half + 1) * LS])
        nc.gpsimd.collective_compute(
            "ReduceScatter", OP.add, replica_groups=[list(range(NCORES))],
            ins=[po[:].opt()], outs=[rso[:].opt()])
        nc.sync.dma_start(out=out_p[:], in_=rso[:])

    nc.compile()
    return nc


def _prep_inputs(inputs):
    """Host-side shard prep: returns in_maps (one dict per core)."""
    h = np.asarray(inputs["hidden"], np.float32)
    W_in = np.asarray(inputs["W_in"], np.float32)
    W_out = np.asarray(inputs["W_out"], np.float32)
    hT = np.ascontiguousarray(h.reshape(B * L, DM).T).astype(BF)

    def f32a(k):
        return np.asarray(inputs[k], np.float32)

    in_maps = []
    for c in range(NCORES):
        sl = slice(c * DC, (c + 1) * DC)
        win = np.concatenate([W_in[sl].T, W_in[DI + c * DC: DI + (c + 1) * DC].T],
                             axis=1)  # (1024, 512): x | z
        m = {
            "hT": hT,
            "win": win.astype(BF),
            "wx": np.stack([f32a("W_x_f")[:, sl].T, f32a("W_x_r")[:, sl].T]).astype(BF),
            "wdt": np.stack([f32a("W_dt_f")[sl].T, f32a("W_dt_r")[sl].T]),
            "wout": W_out[:, sl].T.astype(BF),
            "wconv": np.stack([f32a("conv_w_f")[sl].T, f32a("conv_w_r")[sl].T]),
            "cbias": np.stack([f32a("conv_b_f")[sl], f32a("conv_b_r")[sl]]),
            "bdt": np.stack([-f32a("b_dt_f")[sl], -f32a("b_dt_r")[sl]]),
            "Dp": np.stack([f32a("D_f")[sl], f32a("D_r")[sl]]),
            "Amat": np.stack([np.exp(f32a("A_log_f")[sl]),
                              np.exp(f32a("A_log_r")[sl])]),
        }
        m = {k: np.ascontiguousarray(v) for k, v in m.items()}
        in_maps.append(m)
    return in_maps


def kernel(**inputs) -> np.ndarray:
    from concourse.bass_utils import run_bass_kernel_spmd
    if "nc" not in _CACHE:
        _CACHE["nc"] = _build()
    nc = _CACHE["nc"]
    in_maps = _prep_inputs(inputs)
    res = run_bass_kernel_spmd(nc, in_maps, list(range(NCORES))).results
    # res[c]["out"]: (B, DM, 256) covering t in [256c, 256c+256)
    stripes = np.stack([np.asarray(res[c]["out"], np.float32)
                        for c in range(NCORES)], axis=0)  # (8, B, DM, 256)
    out = stripes.transpose(1, 0, 3, 2).reshape(B, L, DM)
    return np.ascontiguousarray(out)


# revision 16
# speedup vs baseline: 1.0569x; 1.0569x over previous
"""Bidirectional Mamba block on 8 Trainium2 NeuronCores.

Sharding: tensor-parallel over d_inner (2048 -> 256 per core). Each core:
  - computes its x/z in_proj slice from the full hidden (replicated, bf16)
  - depthwise causal conv + silu for both directions (reverse direction is
    computed in flipped coordinates "s-space" so no data flips are needed;
    the reverse scan runs via negative-stride access patterns)
  - x-projection partials -> AllReduce -> full (dt, B, C) per core
  - selective scan per (direction, batch, d-tile, state-index) with the
    hardware tensor_tensor_scan instruction
  - gating + out_proj partial -> ReduceScatter over L-stripes
Host gathers the 8 L-stripes and transposes to (B, L, D_MODEL).

softplus is unavailable in the ACT tables; we use the exact identities
  G = sigmoid(-(x+b)),  delta = softplus(x+b) = -ln G,
  dA_n = exp(A_n * delta) = exp(-A_n * ln G)
so only Sigmoid/Ln/Exp/Silu/Copy LUTs are needed.
"""

from contextlib import ExitStack

import numpy as np
import ml_dtypes

B, L, DM = 2, 2048, 1024
DI, N, DCONV, R = 2048, 16, 4, 64
NCORES = 8
DC = DI // NCORES          # 256 d_inner channels per core
TD = DC // 128             # 2 d-tiles of 128
TC = 512                   # t-chunk for matmuls
NTC = L // TC
BF = ml_dtypes.bfloat16

_CACHE = {}


def _rev(ap):
    """Reverse the (single) free dim of a 2D [partition, free] AP."""
    import concourse.bass as bass
    n = ap.ap[-1][1]
    assert ap.ap[-1][0] == 1
    return bass.AP(ap.tensor, ap.offset + (n - 1),
                   [list(d) for d in ap.ap[:-1]] + [[-1, n]])


def _bcast(ap, parts=128):
    """Partition-broadcast a DRAM row (1D AP of length F) to (parts, F)."""
    import concourse.bass as bass
    assert ap.ap[-1][0] == 1
    return bass.AP(ap.tensor, ap.offset, [[0, parts], [1, ap.ap[-1][1]]])


def _build():
    import concourse.tile as tile
    from concourse import bacc, mybir
    from concourse.masks import make_identity

    f32, bf16 = mybir.dt.float32, mybir.dt.bfloat16
    AF = mybir.ActivationFunctionType
    OP = mybir.AluOpType

    nc = bacc.Bacc("TRN2", target_bir_lowering=False, debug=False,
                   num_devices=NCORES)

    # ---------------- DRAM parameters (per-core shards, host-prepped)
    hT = nc.declare_dram_parameter("hT", [DM, B * L], bf16, isOutput=False)
    win = nc.declare_dram_parameter("win", [DM, 4 * 128], bf16, isOutput=False)
    wx = nc.declare_dram_parameter("wx", [2, DC, 96], bf16, isOutput=False)
    wdt = nc.declare_dram_parameter("wdt", [2, R, DC], f32, isOutput=False)
    wout = nc.declare_dram_parameter("wout", [DC, DM], bf16, isOutput=False)
    wconv = nc.declare_dram_parameter("wconv", [2, DCONV, DC], f32, isOutput=False)
    cbias = nc.declare_dram_parameter("cbias", [2, DC], f32, isOutput=False)
    bdt = nc.declare_dram_parameter("bdt", [2, DC], f32, isOutput=False)  # = -b_dt
    Dp = nc.declare_dram_parameter("Dp", [2, DC], f32, isOutput=False)
    Amat = nc.declare_dram_parameter("Amat", [2, DC, N], f32, isOutput=False)  # = -A
    out_p = nc.declare_dram_parameter("out", [B, DM, L // NCORES], f32,
                                      isOutput=True)

    # ---------------- internal DRAM
    xdp = nc.dram_tensor("xdp", [2, B, 96, L], f32)
    xdr = nc.dram_tensor("xdr", [2, B, 96, L], f32, addr_space="Shared")
    zdram = nc.dram_tensor("zdram", [B * TD, 128, L], bf16)
    udram = nc.dram_tensor("udram", [2, B * TD, 128, L], bf16)
    bcbf = nc.dram_tensor("bcbf", [2, B, 32, L], bf16)
    po = nc.dram_tensor("po", [B, NCORES, DM, L // NCORES], f32)
    rso = nc.dram_tensor("rso", [B, DM, L // NCORES], f32)

    with tile.TileContext(nc) as tc, ExitStack() as es:
        ws = es.enter_context(tc.tile_pool(name="weights", bufs=1))
        wx_sb = ws.tile([128, 2 * TD, 96], bf16)
        nc.sync.dma_start(out=wx_sb[:], in_=wx[:].rearrange("d (t p) m -> p (d t) m", p=128))
        wdt_sb = ws.tile([R, 2, DC], f32)
        nc.sync.dma_start(out=wdt_sb[:], in_=wdt[:].rearrange("d k m -> k d m"))
        wout_sb = ws.tile([128, TD, DM], bf16)
        nc.sync.dma_start(out=wout_sb[:], in_=wout[:].rearrange("(t p) m -> p t m", p=128))
        wconv_sb = ws.tile([128, 2, DCONV, TD], f32)
        nc.sync.dma_start(out=wconv_sb[:], in_=wconv[:].rearrange("d k (t p) -> p d k t", p=128))
        cbias_sb = ws.tile([128, 2, TD], f32)
        nc.sync.dma_start(out=cbias_sb[:], in_=cbias[:].rearrange("d (t p) -> p d t", p=128))
        bdt_sb = ws.tile([128, 2, TD], f32)
        nc.sync.dma_start(out=bdt_sb[:], in_=bdt[:].rearrange("d (t p) -> p d t", p=128))
        Dp_sb = ws.tile([128, 2, TD], f32)
        nc.sync.dma_start(out=Dp_sb[:], in_=Dp[:].rearrange("d (t p) -> p d t", p=128))
        A_sb = ws.tile([128, 2, TD, N], f32)
        nc.sync.dma_start(out=A_sb[:], in_=Amat[:].rearrange("d (t p) n -> p d t n", p=128))

        # persistent SBUF: only comb (out_proj input)
        act = es.enter_context(tc.tile_pool(name="acts", bufs=1))
        comb_sb = [act.tile([128, L], bf16, name=f"comb{g}") for g in range(B * TD)]

        # ---------------- Phase 0/1: in_proj + conv + silu -> udram/zdram
        with tc.tile_pool(name="p01", bufs=2) as p01, \
             tc.tile_pool(name="x01", bufs=1) as x01, \
             tc.tile_pool(name="ps01", bufs=4, space="PSUM") as ps01:
            win_sb = x01.tile([128, 8, 512], bf16)
            nc.sync.dma_start(out=win_sb[:], in_=win[:].rearrange("(k p) m -> p k m", p=128))
            x_sb = [x01.tile([128, L], bf16, name=f"x{g}") for g in range(B * TD)]
            for b in range(B):
                for tcn in range(NTC):
                    hTc = p01.tile([128, 8, TC], bf16, name="hTc")
                    nc.sync.dma_start(
                        out=hTc[:],
                        in_=hT[:, b * L + tcn * TC: b * L + (tcn + 1) * TC]
                        .rearrange("(k p) c -> p k c", p=128))
                    for part in range(4):   # x-dt0, x-dt1, z-dt0, z-dt1
                        pst = ps01.tile([128, TC], f32, name="pst")
                        for k in range(8):
                            nc.tensor.matmul(pst[:], win_sb[:, k, part * 128:(part + 1) * 128],
                                             hTc[:, k, :], start=(k == 0), stop=(k == 7))
                        if part < TD:
                            nc.scalar.copy(
                                out=x_sb[b * TD + part][:, tcn * TC:(tcn + 1) * TC],
                                in_=pst[:])
                        else:
                            zt = p01.tile([128, TC], bf16, name="zt")
                            nc.scalar.activation(out=zt[:], in_=pst[:], func=AF.Silu)
                            nc.sync.dma_start(
                                out=zdram[b * TD + part - TD, :,
                                          tcn * TC:(tcn + 1) * TC],
                                in_=zt[:])
            # depthwise causal conv + silu, both directions
            for d in range(2):
                for g in range(B * TD):
                    ti = g % TD
                    xs = x_sb[g]
                    acc = p01.tile([128, L], bf16, name="acc")
                    if d == 0:  # causal: acc[t] = sum_k w[k]*x[t-3+k] + cb
                        nc.vector.tensor_scalar(
                            out=acc[:], in0=xs[:],
                            scalar1=wconv_sb[:, 0, 3, ti:ti + 1],
                            scalar2=cbias_sb[:, 0, ti:ti + 1],
                            op0=OP.mult, op1=OP.add)
                        for k in (2, 1, 0):
                            s = 3 - k
                            nc.vector.scalar_tensor_tensor(
                                out=acc[:, s:], in0=xs[:, :L - s],
                                scalar=wconv_sb[:, 0, k, ti:ti + 1], in1=acc[:, s:],
                                op0=OP.mult, op1=OP.add)
                    else:  # reverse dir in s-space: acc[s] = sum_m wr[3-m]*x[s+m] + cbr
                        nc.vector.tensor_scalar(
                            out=acc[:], in0=xs[:],
                            scalar1=wconv_sb[:, 1, 3, ti:ti + 1],
                            scalar2=cbias_sb[:, 1, ti:ti + 1],
                            op0=OP.mult, op1=OP.add)
                        for m in (1, 2, 3):
                            nc.vector.scalar_tensor_tensor(
                                out=acc[:, :L - m], in0=xs[:, m:],
                                scalar=wconv_sb[:, 1, 3 - m, ti:ti + 1],
                                in1=acc[:, :L - m],
                                op0=OP.mult, op1=OP.add)
                    ut = p01.tile([128, L], bf16, name="ut")
                    nc.scalar.activation(out=ut[:], in_=acc[:], func=AF.Silu)
                    nc.sync.dma_start(out=udram[d, g], in_=ut[:])

        # ---------------- Phase 2: x-proj partials + AllReduce
        with tc.tile_pool(name="p2", bufs=2) as p2, \
             tc.tile_pool(name="ps2", bufs=4, space="PSUM") as ps2:
            for d in range(2):
                for b in range(B):
                    uts = []
                    for kt in range(TD):
                        ut2 = p2.tile([128, L], bf16, name=f"ut2_{kt}", tag=f"ut2_{kt}")
                        nc.sync.dma_start(out=ut2[:], in_=udram[d, b * TD + kt])
                        uts.append(ut2)
                    for tcn in range(NTC):
                        ps96 = ps2.tile([96, TC], f32, name="ps96")
                        for kt in range(TD):
                            nc.tensor.matmul(ps96[:], wx_sb[:, d * TD + kt, :],
                                             uts[kt][:, tcn * TC:(tcn + 1) * TC],
                                             start=(kt == 0), stop=(kt == TD - 1))
                        sb96 = p2.tile([96, TC], f32, name="sb96")
                        nc.scalar.copy(out=sb96[:], in_=ps96[:])
                        nc.sync.dma_start(
                            out=xdp[d, b, :, tcn * TC:(tcn + 1) * TC], in_=sb96[:])
        # split AllReduce: (b0,d0) first so phase 3 can start early
        for b in range(B):
            for d in range(2):
                nc.gpsimd.collective_compute(
                    "AllReduce", OP.add, replica_groups=[list(range(NCORES))],
                    ins=[xdp[d, b].opt()], outs=[xdr[d, b].opt()])

        # ---------------- Phase 3: delta, scan, gating
        idn = ws.tile([128, 128], bf16, name="idn")
        make_identity(nc, idn[:])
        with tc.tile_pool(name="p3", bufs=1) as p3, \
             tc.tile_pool(name="pbc", bufs=4) as pbc, \
             tc.tile_pool(name="pda", bufs=3) as pda, \
             tc.tile_pool(name="ph", bufs=3) as ph, \
             tc.tile_pool(name="phc", bufs=3) as phc, \
             tc.tile_pool(name="ps3", bufs=2, space="PSUM") as ps3:
            for b in range(B):
                for d in range(2):
                    dtT = p3.tile([R, L], f32, name="dtT", bufs=1)
                    nc.sync.dma_start(out=dtT[:], in_=xdr[d, b, 0:R, :])
                    # stage B/C rows to bf16 (cheap bcast sources)
                    bcst = p3.tile([32, L], f32, name="bcst", bufs=1)
                    nc.sync.dma_start(out=bcst[:], in_=xdr[d, b, 64:96, :])
                    bcsb = p3.tile([32, L], bf16, name="bcsb", bufs=1)
                    nc.scalar.copy(out=bcsb[:], in_=bcst[:])
                    nc.sync.dma_start(out=bcbf[d, b], in_=bcsb[:])
                    lgs, dus, u3s, yaccs = [], [], [], []
                    for ti in range(TD):
                        u3 = p3.tile([128, L], bf16, name=f"u3_{ti}", bufs=1)
                        nc.sync.dma_start(out=u3[:], in_=udram[d, b * TD + ti])
                        u3s.append(u3)
                        # G = sigmoid(-(dtproj + b_dt)); delta = -ln G
                        lg = p3.tile([128, L], f32, name=f"lg{ti}", bufs=1)
                        for tcn in range(NTC):
                            psd = ps3.tile([128, TC], f32, name="psd", tag="ps",
                                           padded_shape=[128, L])
                            nc.tensor.matmul(psd[:], wdt_sb[:, d, ti * 128:(ti + 1) * 128],
                                             dtT[:, tcn * TC:(tcn + 1) * TC],
                                             start=True, stop=True)
                            nc.scalar.copy(out=lg[:, tcn * TC:(tcn + 1) * TC], in_=psd[:])
                        nc.scalar.activation(out=lg[:], in_=lg[:], func=AF.Sigmoid,
                                             scale=-1.0, bias=bdt_sb[:, d, ti:ti + 1])
                        nc.scalar.activation(out=lg[:], in_=lg[:], func=AF.Ln)
                        lgs.append(lg)
                        du = p3.tile([128, L], bf16, name=f"du{ti}", bufs=1)
                        nc.vector.scalar_tensor_tensor(
                            out=du[:], in0=lg[:], scalar=-1.0, in1=u3[:],
                            op0=OP.mult, op1=OP.mult)
                        dus.append(du)
                        yacc = ps3.tile([128, L], f32, name=f"yacc{ti}", tag="ps")
                        yaccs.append(yacc)
                    # scan over the 16 states; PE-accumulated reduction over n
                    for n in range(N):
                        Bbc = pbc.tile([128, L], bf16, name="Bbc")
                        nc.sync.dma_start(out=Bbc[:], in_=_bcast(bcbf[d, b, n, :]))
                        Cbc = pbc.tile([128, L], bf16, name="Cbc")
                        nc.sync.dma_start(out=Cbc[:], in_=_bcast(bcbf[d, b, 16 + n, :]))
                        for ti in range(TD):
                            dA = pda.tile([128, L], f32, name="dA", bufs=4)
                            nc.scalar.activation(out=dA[:], in_=lgs[ti][:], func=AF.Exp,
                                                 scale=A_sb[:, d, ti, n:n + 1])
                            dBu = pda.tile([128, L], bf16, name="dBu", bufs=3)
                            nc.vector.tensor_mul(out=dBu[:], in0=dus[ti][:], in1=Bbc[:])
                            Ht = ph.tile([128, L], bf16, name="Ht", bufs=5)
                            if d == 0:
                                nc.vector.tensor_tensor_scan(
                                    out=Ht[:], data0=dA[:], data1=dBu[:], initial=0.0,
                                    op0=OP.mult, op1=OP.add)
                            else:
                                nc.vector.tensor_tensor_scan(
                                    out=_rev(Ht[:]), data0=_rev(dA[:]), data1=_rev(dBu[:]),
                                    initial=0.0, op0=OP.mult, op1=OP.add)
                            Hc = phc.tile([128, L], bf16, name="Hc", bufs=4)
                            eng_hc = nc.vector if (n % 4 == 3) else nc.gpsimd
                            eng_hc.tensor_mul(out=Hc[:], in0=Ht[:], in1=Cbc[:])
                            for ch in range(NTC):
                                nc.tensor.matmul(
                                    yaccs[ti][:, ch * TC:(ch + 1) * TC], idn[:],
                                    Hc[:, ch * TC:(ch + 1) * TC],
                                    start=(n == 0), stop=(n == N - 1))
                    for ti in range(TD):
                        g = b * TD + ti
                        # gating: comb = (u*D + y) * silu(z)   (zdram holds silu(z))
                        y32 = p3.tile([128, L], f32, name="y32", bufs=1)
                        nc.scalar.copy(out=y32[:], in_=yaccs[ti][:])
                        zt3 = p3.tile([128, L], bf16, name="zt3", bufs=1)
                        nc.sync.dma_start(out=zt3[:], in_=zdram[g])
                        t1 = p3.tile([128, L], bf16, name="t1", bufs=1)
                        nc.vector.scalar_tensor_tensor(
                            out=t1[:], in0=u3s[ti][:], scalar=Dp_sb[:, d, ti:ti + 1],
                            in1=y32[:], op0=OP.mult, op1=OP.add)
                        if d == 0:
                            nc.vector.tensor_mul(out=comb_sb[g][:], in0=t1[:], in1=zt3[:])
                        else:
                            yg = p3.tile([128, L], bf16, name="yg", bufs=1)
                            nc.vector.tensor_mul(out=yg[:], in0=t1[:], in1=zt3[:])
                            nc.vector.tensor_add(out=comb_sb[g][:], in0=comb_sb[g][:],
                                                 in1=yg[:])

        # ---------------- Phase 4: out_proj + ReduceScatter
        LS = L // NCORES  # 256
        with tc.tile_pool(name="ps4", bufs=4, space="PSUM") as ps4, \
             tc.tile_pool(name="p4s", bufs=3) as p4s:
            for b in range(B):
                for mt in range(DM // 128):
                    for tcn in range(NTC):
                        pso = ps4.tile([128, TC], f32, name="pso")
                        for kt in range(TD):
                            nc.tensor.matmul(
                                pso[:], wout_sb[:, kt, mt * 128:(mt + 1) * 128],
                                comb_sb[b * TD + kt][:, tcn * TC:(tcn + 1) * TC],
                                start=(kt == 0), stop=(kt == TD - 1))
                        sbo = p4s.tile([128, TC], f32, name="sbo")
                        nc.scalar.copy(out=sbo[:], in_=pso[:])
                        for half in range(TC // LS):
                            r = tcn * (TC // LS) + half
                            nc.sync.dma_start(
                                out=po[b, r, mt * 128:(mt + 1) * 128, :],
                                in_=sbo[:, half * LS:(half + 1) * LS])
                nc.gpsimd.collective_compute(
                    "ReduceScatter", OP.add, replica_groups=[list(range(NCORES))],
                    ins=[po[b].opt()], outs=[rso[b].opt()])
                nc.sync.dma_start(out=out_p[b], in_=rso[b])

    nc.compile()
    return nc


def _prep_inputs(inputs):
    """Host-side shard prep: returns in_maps (one dict per core)."""
    h = np.asarray(inputs["hidden"], np.float32)
    W_in = np.asarray(inputs["W_in"], np.float32)
    W_out = np.asarray(inputs["W_out"], np.float32)
    hT = np.ascontiguousarray(h.reshape(B * L, DM).T).astype(BF)

    def f32a(k):
        return np.asarray(inputs[k], np.float32)

    in_maps = []
    for c in range(NCORES):
        sl = slice(c * DC, (c + 1) * DC)
        win = np.concatenate([W_in[sl].T, W_in[DI + c * DC: DI + (c + 1) * DC].T],
                             axis=1)  # (1024, 512): x | z
        m = {
            "hT": hT,
            "win": win.astype(BF),
            "wx": np.stack([f32a("W_x_f")[:, sl].T, f32a("W_x_r")[:, sl].T]).astype(BF),
            "wdt": np.stack([f32a("W_dt_f")[sl].T, f32a("W_dt_r")[sl].T]),
            "wout": W_out[:, sl].T.astype(BF),
            "wconv": np.stack([f32a("conv_w_f")[sl].T, f32a("conv_w_r")[sl].T]),
            "cbias": np.stack([f32a("conv_b_f")[sl], f32a("conv_b_r")[sl]]),
            "bdt": np.stack([-f32a("b_dt_f")[sl], -f32a("b_dt_r")[sl]]),
            "Dp": np.stack([f32a("D_f")[sl], f32a("D_r")[sl]]),
            "Amat": np.stack([np.exp(f32a("A_log_f")[sl]),
                              np.exp(f32a("A_log_r")[sl])]),
        }
        m = {k: np.ascontiguousarray(v) for k, v in m.items()}
        in_maps.append(m)
    return in_maps


def kernel(**inputs) -> np.ndarray:
    from concourse.bass_utils import run_bass_kernel_spmd
    if "nc" not in _CACHE:
        _CACHE["nc"] = _build()
    nc = _CACHE["nc"]
    in_maps = _prep_inputs(inputs)
    res = run_bass_kernel_spmd(nc, in_maps, list(range(NCORES))).results
    # res[c]["out"]: (B, DM, 256) covering t in [256c, 256c+256)
    stripes = np.stack([np.asarray(res[c]["out"], np.float32)
                        for c in range(NCORES)], axis=0)  # (8, B, DM, 256)
    out = stripes.transpose(1, 0, 3, 2).reshape(B, L, DM)
    return np.ascontiguousarray(out)
